# revision 30
# baseline (speedup 1.0000x reference)
"""Trainium2 Bass kernel for nn_MultiHeadAttention_60851096649901.

Sharding: 8 cores = 4 batches x 2 head-groups (8 heads each).
Each core computes its batch's attention for its 8 heads plus the partial
out-projection; host sums the two head-group partials and adds bo.

Per-core math (fp16 q/k path -- 16-bit PE streaming is 2x fp32's rate):
  qT/kT = (Wg.T @ x.T + b)  fp16 [128, 2048] per head-pair (d-major)
  v_sb  = (x @ Wv + bv) in bf16, 65-col/head layout (col 64 = 1.0)
  attention pair-packed: per head-pair, q-chunk of 512, k-tile of 128:
    scoresT[k, qA|qB] via concurrent row-group fp16 matmuls (PSUM f32)
    et = exp(8*s - 100) in ONE [128,1024] ACT instr -> bf16 SBUF
    pv[65, qA|qB] += v_sb.T @ et  (row 64 = softmax denominator)
  (fp16 for q/k keeps rel err ~6e-3; bf16 there blew past the 2e-2 gate.
   et/reciprocals need bf16/f32 RANGE -- exp values span e-80..e+64.)

Scheduling (v3): one dependency-paced stream.
  - Attention starts ~10us in: only the first kT/qT 512-token chunks are
    emitted upfront; ALL remaining projections (q/k chunks, the entire V
    projection) are demand-pulled fillers (deque + need(key)) inside
    pair-0's attention, overlapping v1's 90us serial prologue with exp/PV.
  - Software-pipelined inner loop: PV trails scores by one k-tile so
    scores(kt+1) reaches the PE queue before PV(kt) blocks on ACT(kt);
    keeps the exp stream back-to-back and HAM at K=8/8 (v1 was cold 41%).
  - Deferred normalization: denominator row copied to partition 0 (a
    standard-op copy bridges partition bases; the custom-DVE
    reciprocal_approx_fast crashes on mismatched bases), then one approx
    reciprocal per chunk (~5x cheaper than the iterative reciprocal that
    ate 106us of DVE in v1 and head-of-line-blocked the outproj chain).
  - PSUM: score ring 2x[128,1024] (4 banks) + unified pv [65,1024]
    (2 banks) + filler ring 2x[128,512] (2 banks) = 8 banks exactly.
"""

import numpy as np
from collections import deque

S = 2048
E = 1024
D = 64
P = 128
HCORE = 8          # heads per core
NPAIR = 4          # head-pairs per core
C_OFF = 100.0      # softmax constant offset (exp(8*s - C))
INV_SCALE = 8.0    # sqrt(head_dim)

_BUILT = None


def _build():
    import concourse.bass as bass
    import concourse.tile as tile
    from concourse import bacc, mybir

    f32 = mybir.dt.float32
    f32r = mybir.dt.float32r
    bf16 = mybir.dt.bfloat16
    fp16 = mybir.dt.float16
    Exp = mybir.ActivationFunctionType.Exp

    nc = bacc.Bacc("TRN2", target_bir_lowering=False, debug=False, num_devices=8)

    xT_d = nc.dram_tensor("xT", [P, 8, S], fp16, kind="ExternalInput")
    wq_d = nc.dram_tensor("wq", [4, P, 8, P], fp16, kind="ExternalInput")
    wk_d = nc.dram_tensor("wk", [4, P, 8, P], fp16, kind="ExternalInput")
    bq_d = nc.dram_tensor("bq", [4, P, 1], f32, kind="ExternalInput")
    bk_d = nc.dram_tensor("bk", [4, P, 1], f32, kind="ExternalInput")
    wv_d = nc.dram_tensor("wv", [E, 512], fp16, kind="ExternalInput")
    wo_d = nc.dram_tensor("wo", [512, E], bf16, kind="ExternalInput")
    y_d = [
        nc.dram_tensor(f"y{jt}", [S, E], f32, kind="ExternalOutput")
        for jt in range(NPAIR)
    ]

    with tile.TileContext(nc) as tc:
        with (
            tc.tile_pool(name="persist", bufs=1) as persist,
            tc.tile_pool(name="wpool", bufs=2) as wpool,
            tc.tile_pool(name="qk", bufs=2) as qkpool,
            tc.tile_pool(name="att", bufs=4) as att,
            tc.tile_pool(name="norm", bufs=3) as norm,
            tc.tile_pool(name="rcp", bufs=2) as rcp,
            tc.tile_pool(name="oh", bufs=2) as ohpool,
            tc.tile_pool(name="yout", bufs=4) as yout,
            tc.tile_pool(name="sc", bufs=2, space="PSUM") as scps,    # scores
            tc.tile_pool(name="fl", bufs=2, space="PSUM") as fps,     # fillers
            tc.tile_pool(name="pv", bufs=1, space="PSUM") as pvps,    # pv A|B
        ):
            # ---- tiny warm-up ACT so the exp table set loads at t~0 --------
            warm = persist.tile([1, 16], f32, tag="warm")
            nc.vector.memset(warm[:], 0.0)
            nc.scalar.activation(out=warm[:], in_=warm[:], func=Exp)

            # PE warm-up spin: ~6us of dummy matmuls during the initial DMA
            # wait flips HAM to K=8/8 before the first real matmul
            wmm = persist.tile([P, 512], fp16, tag="wmm")
            nc.vector.memset(wmm[:], 0.0)
            for _ in range(8):
                wps = fps.tile([P, 512], f32, tag="f", name="warmps")
                nc.tensor.matmul(wps[:], wmm[:, 0:P], wmm[:],
                                 start=True, stop=False)
                nc.tensor.matmul(wps[:], wmm[:, 0:P], wmm[:],
                                 start=False, stop=True)

            neg_c = persist.tile([P, 1], f32, tag="neg_c")
            nc.vector.memset(neg_c[:], -C_OFF)

            # ---- persistent loads -------------------------------------------
            ones_bb = persist.tile([1, 64], bf16, tag="ones_bb")
            nc.vector.memset(ones_bb[:], 1.0)

            # v_sb: [k-part, k-tile, head, 65]; col 64 = 1.0 (denominator)
            v_sb = persist.tile([P, 16, 8, 65], bf16, tag="v_sb")
            nc.vector.memset(v_sb[:, :, :, 64:65], 1.0)

            xT = persist.tile([P, 8, S], fp16, tag="xT")  # [i-part, i-tile, q]

            def load_pair_weights(jt):
                js = slice(jt * P, (jt + 1) * P)
                wq = wpool.tile([P, 8, P], fp16, tag="wq", name=f"wq{jt}")
                wk = wpool.tile([P, 8, P], fp16, tag="wk", name=f"wk{jt}")
                nc.sync.dma_start(wk[:, :, :], wk_d[jt])
                nc.sync.dma_start(wq[:, :, :], wq_d[jt])
                bqr = wpool.tile([P, 1], f32, tag="bqr", name=f"bqr{jt}")
                bkr = wpool.tile([P, 1], f32, tag="bkr", name=f"bkr{jt}")
                nc.sync.dma_start(bqr[:], bq_d[jt])
                nc.sync.dma_start(bkr[:], bk_d[jt])
                wo = wpool.tile([P, E], bf16, tag="wo", name=f"wo{jt}")
                nc.sync.dma_start(wo[:], wo_d[js, :])
                return wq, wk, bqr, bkr, wo

            # DMA order: xT chunk 0 first (unblocks the first projections),
            # then pair-0 weights, wv, the rest of xT, pair-1 weights
            def load_xt(cc):
                nc.sync.dma_start(
                    xT[:, :, cc * 512:(cc + 1) * 512],
                    xT_d[:, :, cc * 512:(cc + 1) * 512],
                )

            load_xt(0)
            wq0 = wpool.tile([P, 8, P], fp16, tag="wq", name="wq0")
            wk0 = wpool.tile([P, 8, P], fp16, tag="wk", name="wk0")
            nc.sync.dma_start(wk0[:, :, :], wk_d[0])
            nc.sync.dma_start(wq0[:, :, :], wq_d[0])
            bqr0 = wpool.tile([P, 1], f32, tag="bqr", name="bqr0")
            bkr0 = wpool.tile([P, 1], f32, tag="bkr", name="bkr0")
            nc.sync.dma_start(bqr0[:], bq_d[0])
            nc.sync.dma_start(bkr0[:], bk_d[0])
            wo0 = wpool.tile([P, E], bf16, tag="wo", name="wo0")
            nc.sync.dma_start(wo0[:], wo_d[0:P, :])
            pw = {0: (wq0, wk0, bqr0, bkr0, wo0)}
            load_xt(1)
            wv = persist.tile([P, 8, 512], fp16, tag="wv")
            for i in range(8):
                nc.sync.dma_start(wv[:, i, :], wv_d[i * P:(i + 1) * P, :])
            pw[1] = load_pair_weights(1)
            load_xt(2)
            load_xt(3)

            # ---- demand-driven op queue ------------------------------------
            Q = deque()        # (key, fn) dependency-ordered filler ops
            done_keys = set()  # keys fully emitted
            key_counts = {}

            def q_push(key, fns):
                key_counts[key] = len(fns)
                for fn in fns:
                    Q.append((key, fn))

            def q_pop_one():
                key, fn = Q.popleft()
                fn()
                key_counts[key] -= 1
                if key_counts[key] == 0:
                    done_keys.add(key)

            def need(key):
                while key not in done_keys and Q:
                    q_pop_one()

            # ---- op generators ---------------------------------------------
            def v_chunk_ops(kt):
                """v_sb[:, kt] = x @ Wv + bv (9 MMs + strided bf16 evict)."""
                st = {}
                ops = []

                def mk_mm(i):
                    def op():
                        if i == 0:
                            st["p"] = fps.tile([P, 512], f32, tag="f",
                                               name=f"vps{kt}")
                        nc.tensor.matmul(
                            st["p"][:], xT[:, i, kt * P:(kt + 1) * P],
                            wv[:, i, :], start=(i == 0), stop=(i == 7),
                        )
                    return op

                for i in range(8):
                    ops.append(mk_mm(i))

                def fin():
                    # bv is folded host-side: softmax weights sum to 1, so
                    # out = p.T@v + bv and bv@Wo is a constant added on host
                    nc.vector.tensor_copy(v_sb[:, kt, :, 0:64], st["p"][:])
                ops.append(fin)
                return ops

            def proj_chunk_ops(w, br, dst, c4, nm):
                """qT/kT token-chunk c4 (512 wide): 8 MMs + bias + evict."""
                st = {}
                ops = []
                ts = slice(c4 * 512, (c4 + 1) * 512)

                def mk_mm(i):
                    def op():
                        if i == 0:
                            st["p"] = fps.tile([P, 512], f32, tag="f",
                                               name=f"pp{nm}")
                        nc.tensor.matmul(
                            st["p"][:], w[:, i, :], xT[:, i, ts],
                            start=(i == 0), stop=(i == 7),
                        )
                    return op

                for i in range(8):
                    ops.append(mk_mm(i))

                def fin():
                    # bias folded into the eviction: per-partition scalar add
                    nc.vector.tensor_scalar_add(dst[:, ts], st["p"][:], br[:])
                ops.append(fin)
                return ops

            def make_pair_proj(jt):
                """Allocate qT/kT tiles; push per-chunk ops with demand keys."""
                wq, wk, bqr, bkr, wo = pw[jt]
                qT = qkpool.tile([P, S], fp16, tag="qT", name=f"qT{jt}")
                kT = qkpool.tile([P, S], fp16, tag="kT", name=f"kT{jt}")
                chunks = []
                for c4 in range(4):
                    chunks.append((f"k{jt}_{c4}",
                                   proj_chunk_ops(wk, bkr, kT, c4, f"k{jt}{c4}")))
                for c4 in range(4):
                    chunks.append((f"q{jt}_{c4}",
                                   proj_chunk_ops(wq, bqr, qT, c4, f"q{jt}{c4}")))
                return qT, kT, chunks

            def norm_outproj_chunk_ops(g, jt, qc, pvc, outhT, wo):
                last = (jt == NPAIR - 1 and qc == 3)
                """Deferred normalization + out-projection for (pair, chunk).

                reciprocal_approx_fast (1 DVE op, ~0.7us) replaces the v1
                iterative reciprocal (3.3us).  Ops release (unlock_iter, fn)
                so nothing head-of-line-blocks its engine queue; everything
                lands within the next chunk's iterations.
                """
                qs = slice(qc * 512, (qc + 1) * 512)
                st = {}
                ops = []

                def mk_dn():
                    def op():
                        # standard-op copy bridges partition base 64 -> 0;
                        # the custom-DVE reciprocal requires matching bases
                        dn = rcp.tile([1, 1024], f32, tag="dn",
                                      name=f"dn{jt}_{qc}")
                        nc.vector.tensor_copy(dn[:], pvc[64:65, :])
                        st["dn"] = dn
                    return op

                def mk_recip():
                    def op():
                        rc = rcp.tile([1, 1024], f32, tag="rc",
                                      name=f"rc{jt}_{qc}")
                        nc.vector.reciprocal_approx_fast(rc[:], st["dn"][:])
                        rcr = rcp.tile([1, 1024], bf16, tag="rcr",
                                       name=f"rcr{jt}_{qc}")
                        nc.vector.tensor_copy(rcr[:], rc[:])
                        st["rcr"] = rcr
                    return op

                def mk_bcmul(h2):
                    def op():
                        hb = h2 * 64
                        bc = fps.tile([64, 512], f32, tag="f",
                                      name=f"bc{jt}_{qc}_{h2}")
                        nc.tensor.matmul(bc[:], ones_bb[:],
                                         st["rcr"][:, h2 * 512:(h2 + 1) * 512],
                                         start=True, stop=True)
                        nc.vector.tensor_mul(
                            outhT[hb:hb + 64, qs],
                            pvc[0:64, h2 * 512:(h2 + 1) * 512], bc[:])
                    return op

                ops.append((g + 1, mk_dn()))
                ops.append((g + 2, mk_recip()))
                ops.append((g + 4, mk_bcmul(0)))
                ops.append((g + 6, mk_bcmul(1)))

                def mk_opqt(qt, e):
                    def op():
                        yp = fps.tile([P, 512], f32, tag="f",
                                      name=f"yps{jt}_{qt}_{e}")
                        nc.tensor.matmul(
                            yp[:],
                            outhT[:, qt * P:(qt + 1) * P],
                            wo[:, e * 512:(e + 1) * 512],
                            start=True, stop=True,
                        )
                        ysb = yout.tile([P, 512], f32, tag="ysb",
                                        name=f"ysb{jt}_{qt}_{e}")
                        if last and e == 1:
                            nc.scalar.copy(ysb[:], yp[:])
                        else:
                            nc.vector.tensor_copy(ysb[:], yp[:])
                        nc.sync.dma_start(
                            y_d[jt][qt * P:(qt + 1) * P,
                                    e * 512:(e + 1) * 512],
                            ysb[:])
                    return op

                k = 0
                base = 5 if jt == NPAIR - 1 else 8
                for qt in range(qc * 4, qc * 4 + 4):
                    for e in range(2):
                        ops.append((g + base + k, mk_opqt(qt, e)))
                        k += 1
                return ops

            # ---- upfront: just enough to start attention --------------------
            qk = {}
            qT0, kT0, chunks0 = make_pair_proj(0)
            qk[0] = (qT0, kT0)
            # emit kT chunk 0 + qT chunk 0 inline (~4us of PE work)
            for key, fns in chunks0[:1] + chunks0[4:5]:
                for fn in fns:
                    fn()
                done_keys.add(key)
            # rest of pair-0 projections + full V projection: demand fillers
            q_push("k0_1", chunks0[1][1])
            for kt in range(2):
                q_push(f"v{kt}", v_chunk_ops(kt))
            q_push("k0_2", chunks0[2][1])
            for kt in range(2, 6):
                q_push(f"v{kt}", v_chunk_ops(kt))
            q_push("k0_3", chunks0[3][1])
            for kt in range(6, 16):
                q_push(f"v{kt}", v_chunk_ops(kt))
            for key, fns in chunks0[5:]:
                q_push(key, fns)

            # ---- attention per pair, latency-scheduled fillers --------------
            L = deque()   # (unlock_iter, op): deferred norm + outproj
            git = 0
            NIT = NPAIR * 64
            for jt in range(NPAIR):
                qT, kT = qk[jt]
                wo_cur = pw[jt][4]

                if jt + 1 < NPAIR:
                    qTn, kTn, chunksn = make_pair_proj(jt + 1)
                    qk[jt + 1] = (qTn, kTn)
                    for key, fns in chunksn:
                        q_push(key, fns)

                outhT = ohpool.tile([P, S], bf16, tag="outhT", name=f"oh{jt}")
                for qc in range(4):
                    qs = slice(qc * 512, (qc + 1) * 512)
                    need(f"q{jt}_{qc}")
                    if qc == 2 and jt + 2 < NPAIR:
                        # prefetch pair jt+2 weights: ring slot of pair jt --
                        # force-emit all pair-jt projection readers first
                        for c4 in range(4):
                            need(f"k{jt}_{c4}")
                            need(f"q{jt}_{c4}")
                        pw[jt + 2] = load_pair_weights(jt + 2)
                    pv = pvps.tile([65, 1024], f32, tag="pv",
                                   name=f"pv{jt}_{qc}")

                    def emit_pv(k_t, et):
                        for h2 in range(2):
                            h = jt * 2 + h2
                            nc.tensor.matmul(
                                pv[:, h2 * 512:(h2 + 1) * 512],
                                v_sb[:, k_t, h, :],
                                et[:, h2 * 512:(h2 + 1) * 512],
                                start=(k_t == 0), stop=(k_t == 15),
                            )

                    # software-pipelined: PV trails scores by one iteration so
                    # scores(kt+1) reaches the PE queue before PV(kt) blocks
                    # on ACT(kt) -- keeps ACT back-to-back
                    pend = None
                    for k_t in range(16):
                        sc = scps.tile([P, 1024], f32, tag="sc")
                        for h2 in range(2):
                            hb = h2 * 64
                            nc.tensor.matmul(
                                sc[:, h2 * 512:(h2 + 1) * 512],
                                kT[hb:hb + 64, k_t * P:(k_t + 1) * P],
                                qT[hb:hb + 64, qs],
                                start=True, stop=True,
                            )
                        et = att.tile([P, 1024], bf16, tag="exp")
                        nc.scalar.activation(
                            out=et[:], in_=sc[:], func=Exp,
                            bias=neg_c[:], scale=INV_SCALE,
                        )
                        git += 1
                        need(f"v{k_t}")  # PV(k_t) weights, consumed next iter
                        # early filler pops: execute inside PV's
                        # queue-head wait for ACT(k_t-1) completion
                        early = 0
                        for _ in range(2):
                            if L and L[0][0] <= git:
                                L.popleft()[1]()
                                early += 1
                            elif Q:
                                q_pop_one()
                                early += 1
                            else:
                                break
                        if pend is not None:
                            emit_pv(*pend)
                        pend = (k_t, et)
                        # demand-prefetch next iterations' kT chunk
                        need(f"k{jt}_{min(k_t + 2, 15) // 4}")
                        # fillers: keep the PE fed without starving ACT
                        backlog = len(Q) + len(L)
                        budget = max(0, (3 if backlog > 2 * (NIT - git)
                                         else 2) - early)
                        for _ in range(budget):
                            if L and L[0][0] <= git:
                                L.popleft()[1]()
                            elif Q:
                                q_pop_one()
                            else:
                                break
                    emit_pv(*pend)
                    # evict PV out of PSUM now; defer normalization + outproj
                    pvc = norm.tile([65, 1024], f32, tag="pvc",
                                    name=f"pvc{jt}_{qc}")
                    # two half-copies: next chunk's PV-A only WARs on the
                    # first half's eviction (subtile deps), halving the
                    # chunk-boundary pv-reuse latency
                    nc.vector.tensor_copy(pvc[:, 0:512], pv[:, 0:512])
                    nc.vector.tensor_copy(pvc[:, 512:1024], pv[:, 512:1024])
                    L.extend(norm_outproj_chunk_ops(
                        git, jt, qc, pvc, outhT, wo_cur))

            # drain remaining deferred work (last pair's norm + outproj)
            while L or Q:
                if L:
                    L.popleft()[1]()
                elif Q:
                    q_pop_one()

    nc.compile()
    return nc


def _get_nc():
    global _BUILT
    if _BUILT is None:
        _BUILT = _build()
    return _BUILT


def _prep_core_inputs(x, Wq, bq, Wk, bk, Wv, bv, Wo, g, b):
    gs = g * 512
    bf = 'bfloat16'
    fp = np.float16
    xT = np.ascontiguousarray(
        x[b].T.astype(fp).reshape(8, P, S).transpose(1, 0, 2))
    wq = np.ascontiguousarray(
        Wq[:, gs:gs + 512].astype(fp).reshape(8, P, 4, P).transpose(2, 1, 0, 3))
    wk = np.ascontiguousarray(
        Wk[:, gs:gs + 512].astype(fp).reshape(8, P, 4, P).transpose(2, 1, 0, 3))
    bqs = np.ascontiguousarray(
        bq[gs:gs + 512].astype(np.float32).reshape(4, P, 1))
    bks = np.ascontiguousarray(
        bk[gs:gs + 512].astype(np.float32).reshape(4, P, 1))
    wv = np.ascontiguousarray(Wv[:, gs:gs + 512].astype(fp))
    bvo = np.ones((1, 1024), np.float32)
    bvo[0, 0:512] = bv[gs:gs + 512]
    bvo = bvo.astype(fp)
    ones = np.ones((1, 64), np.float32)
    wo = np.ascontiguousarray(Wo[gs:gs + 512, :].astype(bf))
    return {
        "xT": xT, "wq": wq, "wk": wk, "bq": bqs, "bk": bks,
        "wv": wv, "bvo": bvo, "ones": ones, "wo": wo,
    }


def kernel(x, Wq, bq, Wk, bk, Wv, bv, Wo, bo):
    from concourse.bass_utils import run_bass_kernel_spmd

    x = np.asarray(x)
    B = x.shape[0]
    nc = _get_nc()
    in_maps = []
    for c in range(8):
        g, b = c // 4, c % 4
        in_maps.append(
            _prep_core_inputs(x, np.asarray(Wq), np.asarray(bq), np.asarray(Wk),
                              np.asarray(bk), np.asarray(Wv), np.asarray(bv),
                              np.asarray(Wo), g, b)
        )
    res = run_bass_kernel_spmd(nc, in_maps, list(range(8)))
    y = np.zeros((B, S, E), np.float32)
    bo = np.asarray(bo, dtype=np.float32)
    for c in range(8):
        b = c % 4
        for jt in range(NPAIR):
            y[b] += res.results[c][f"y{jt}"]
    # bv contribution: softmax weights sum to 1 -> + bv @ Wo (host-side)
    y += bo + np.asarray(bv, dtype=np.float32) @ np.asarray(Wo, dtype=np.float32)
    return y
